# revision 4
# baseline (speedup 1.0000x reference)
"""MHA (B=2, S=2048, D=1024, H=16) on 8 Trainium2 NeuronCores.

Sharding: core c = (batch b = c//4, head-group g = c%4, 4 heads each).
Per core: Q/K/V projections for its 4 heads (tensor-parallel column split),
full attention for those heads in transposed (scoresT = [k, q]) layout so no
on-chip transposes are needed, then an 8-way AllToAll redistributes the
per-head outputs so each core owns a 256-row sequence slice of BOTH batches
with all 1024 concat-head dims, and runs the output projection for its slice.
Host only concatenates slices.

All matmuls run as float32r (FP22 truncated reads — full PE rate, ~1e-4 rel
err). Softmax skips max-subtraction (scores ~N(0,1), fp32 exp is safe) and
folds the row-sum into the attention@V matmul via an extra all-ones column on
V; normalization happens after, via partition_broadcast + reciprocal.
"""

import numpy as np

B, S, D, H = 2, 2048, 1024, 16
HD = D // H          # 64
G = H // 4           # 4 head-groups
GH = 4               # heads per core
LD = GH * HD         # 256 local dims per core
CORES = 8
QS = S // CORES      # 256: per-core final sequence slice (per batch)
P = 128
NB = 4               # 2048 / 512 column blocks
KT = D // P          # 8 k-tiles of the model dim

_CACHE = {}


def _build_nc():
    import concourse.mybir as mybir
    import concourse.tile as tile
    from concourse import bacc

    F32 = mybir.dt.float32
    F32R = mybir.dt.float32r
    EXP = mybir.ActivationFunctionType.Exp
    MUL = mybir.AluOpType.mult
    ADD = mybir.AluOpType.add

    nc = bacc.Bacc("TRN2", target_bir_lowering=False, debug=False,
                   num_devices=CORES)

    d_xT = nc.dram_tensor("xT", [D, S], F32, kind="ExternalInput")
    d_wq = nc.dram_tensor("wqT", [D, LD], F32, kind="ExternalInput")
    d_wk = nc.dram_tensor("wkT", [D, LD], F32, kind="ExternalInput")
    d_wv = nc.dram_tensor("wvT", [D, LD], F32, kind="ExternalInput")
    d_bq = nc.dram_tensor("bq2", [P, 2], F32, kind="ExternalInput")
    d_bk = nc.dram_tensor("bk2", [P, 2], F32, kind="ExternalInput")
    d_vi = nc.dram_tensor("vinit", [1, GH * (HD + 1)], F32, kind="ExternalInput")
    d_wo = nc.dram_tensor("woT", [D, D], F32, kind="ExternalInput")
    d_bo = nc.dram_tensor("bo1", [1, D], F32, kind="ExternalInput")
    d_y = nc.dram_tensor("y", [B, QS, D], F32, kind="ExternalOutput")

    VW = HD + 1  # 65: per-head V width incl. ones column

    with tile.TileContext(nc) as tc:
        with (
            tc.tile_pool(name="statics", bufs=1) as st,
            tc.tile_pool(name="dram", bufs=1, space="DRAM") as dram,
        ):
            # ---- small persistent statics ----
            bq = st.tile([P, 2], F32, tag="bq", name="bq")
            bk = st.tile([P, 2], F32, tag="bk", name="bk")
            nc.sync.dma_start(bq[:], d_bq[:])
            nc.sync.dma_start(bk[:], d_bk[:])
            vib = st.tile([P, GH * VW], F32, tag="vib", name="vib")
            bob = st.tile([P, D], F32, tag="bob", name="bob")

            # persistent activations
            qT = [st.tile([P, S], F32R, tag=f"qT{t}", name=f"qT{t}") for t in range(2)]
            kTt = [st.tile([P, S], F32R, tag=f"kT{t}", name=f"kT{t}") for t in range(2)]
            vaug = [st.tile([P, GH * VW], F32R, tag=f"va{s}", name=f"va{s}") for s in range(S // P)]
            outT = [st.tile([P, S], F32R, tag=f"oT{t}", name=f"oT{t}") for t in range(2)]

            with (
                tc.tile_pool(name="xTp", bufs=1) as xp,
                tc.tile_pool(name="pj", bufs=3, space="PSUM") as pj,
                tc.tile_pool(name="pv", bufs=2, space="PSUM") as pv,
            ):
                wq = [xp.tile([P, LD], F32R, tag=f"wq{k}", name=f"wq{k}") for k in range(KT)]
                wk = [xp.tile([P, LD], F32R, tag=f"wk{k}", name=f"wk{k}") for k in range(KT)]
                wv = [xp.tile([P, LD], F32R, tag=f"wv{k}", name=f"wv{k}") for k in range(KT)]
                for k in range(KT):
                    nc.sync.dma_start(wq[k][:], d_wq[k * P:(k + 1) * P, :].bitcast(F32R))
                    nc.sync.dma_start(wk[k][:], d_wk[k * P:(k + 1) * P, :].bitcast(F32R))
                    nc.sync.dma_start(wv[k][:], d_wv[k * P:(k + 1) * P, :].bitcast(F32R))
                vi1 = xp.tile([1, GH * VW], F32, tag="vi1", name="vi1")
                nc.sync.dma_start(vi1[:], d_vi[:])
                nc.gpsimd.partition_broadcast(vib[:], vi1[:])
                bo1 = xp.tile([1, D], F32, tag="bo1", name="bo1")
                nc.sync.dma_start(bo1[:], d_bo[:])
                nc.gpsimd.partition_broadcast(bob[:], bo1[:])
                xT = [xp.tile([P, S], F32R, tag=f"xT{k}", name=f"xT{k}") for k in range(KT)]
                for k in range(KT):
                    nc.sync.dma_start(xT[k][:], d_xT[k * P:(k + 1) * P, :].bitcast(F32R))

                # ---- Q/K projections (transposed layout [256, 2048]) ----
                for (ws, bs, dst) in ((wq, bq, qT), (wk, bk, kTt)):
                    for m in range(2):
                        for n in range(NB):
                            ps = pj.tile([P, 512], F32, tag="pj", name="pj")
                            for k in range(KT):
                                nc.tensor.matmul(
                                    ps[:], ws[k][:, m * P:(m + 1) * P],
                                    xT[k][:, n * 512:(n + 1) * 512],
                                    start=(k == 0), stop=(k == KT - 1))
                            nc.vector.tensor_tensor(
                                dst[m][:, n * 512:(n + 1) * 512], ps[:],
                                bs[:, m:m + 1].to_broadcast((P, 512)), ADD)

                # ---- V projection (natural layout, head-interleaved + ones col) ----
                for s in range(S // P):
                    ps = pv.tile([P, LD], F32, tag="pv", name="pv")
                    for k in range(KT):
                        nc.tensor.matmul(
                            ps[:], xT[k][:, s * P:(s + 1) * P], wv[k][:],
                            start=(k == 0), stop=(k == KT - 1))
                    va = vaug[s].rearrange("p (h w) -> p h w", w=VW)
                    vb = vib.rearrange("p (h w) -> p h w", w=VW)
                    nc.vector.tensor_tensor(
                        va[:, :, 0:HD], ps.rearrange("p (h w) -> p h w", w=HD),
                        vb[:, :, 0:HD], ADD)
                    nc.vector.tensor_copy(va[:, :, HD:VW], vb[:, :, HD:VW])

            # ---- attention, per head, scoresT = [k_pos, q_pos] layout ----
            with (
                tc.tile_pool(name="att", bufs=3) as at,
                tc.tile_pool(name="nrm", bufs=1) as nr,
                tc.tile_pool(name="psc", bufs=2, space="PSUM") as psc,
                tc.tile_pool(name="pav", bufs=1, space="PSUM") as pav,
                tc.tile_pool(name="late", bufs=1) as lt,
            ):
                wo = [lt.tile([P, D], F32R, tag=f"wo{k}", name=f"wo{k}") for k in range(KT)]
                for k in range(KT):
                    nc.sync.dma_start(wo[k][:], d_wo[k * P:(k + 1) * P, :].bitcast(F32R))

                for h in range(GH):
                    ht, hp = h // 2, (h % 2) * HD
                    kTh = kTt[ht][hp:hp + HD, :]
                    qTh = qT[ht][hp:hp + HD, :]
                    av = pav.tile([VW, S], F32, tag="av", name="av")
                    for kb in range(S // P):
                        ex = at.tile([P, S], F32R, tag="ex", name="ex")
                        for half in range(2):
                            sc = psc.tile([P, 1024], F32, tag="sc", name="sc")
                            for nn in range(2):
                                qc = half * 1024 + nn * 512
                                nc.tensor.matmul(
                                    sc[:, nn * 512:(nn + 1) * 512],
                                    kTh[:, kb * P:(kb + 1) * P],
                                    qTh[:, qc:qc + 512],
                                    start=True, stop=True)
                            nc.scalar.activation(
                                ex[:, half * 1024:(half + 1) * 1024], sc[:], EXP)
                        for qb in range(NB):
                            nc.tensor.matmul(
                                av[:, qb * 512:(qb + 1) * 512],
                                vaug[kb][:, h * VW:(h + 1) * VW],
                                ex[:, qb * 512:(qb + 1) * 512],
                                start=(kb == 0), stop=(kb == S // P - 1))
                    # normalize: outT_h = av[0:64] / av[64]
                    rs = nr.tile([1, S], F32, tag="rs", name="rs")
                    nc.vector.tensor_copy(rs[:], av[HD:VW, :])
                    rsb = nr.tile([HD, S], F32, tag="rsb", name="rsb")
                    nc.gpsimd.partition_broadcast(rsb[:], rs[:])
                    nc.vector.reciprocal(rsb[:], rsb[:])
                    nc.vector.tensor_tensor(
                        outT[ht][hp:hp + HD, :], av[0:HD, :], rsb[:], MUL)

                # ---- 8-way AllToAll: chunk j = outT[:, j*256:(j+1)*256] ----
                a_in = dram.tile([CORES * LD, QS], F32, name="a_in")
                a_out = dram.tile([CORES * LD, QS], F32, name="a_out")
                for j in range(CORES):
                    for t in range(2):
                        nc.sync.dma_start(
                            a_in[j * LD + t * P: j * LD + (t + 1) * P, :],
                            outT[t][:, j * QS:(j + 1) * QS].bitcast(F32))
                nc.gpsimd.collective_compute(
                    "AllToAll",
                    mybir.AluOpType.bypass,
                    replica_groups=[list(range(CORES))],
                    ins=[a_in[:]],
                    outs=[a_out[:]],
                )

                # ---- out projection for my 256-row slice of each batch ----
                ao = [lt.tile([P, QS], F32R, tag=f"ao{i}", name=f"ao{i}") for i in range(2 * KT)]
                for i in range(2 * KT):
                    nc.sync.dma_start(
                        ao[i][:], a_out[i * P:(i + 1) * P, :].bitcast(F32R))
                for bb in range(B):
                    for m in range(QS // P):
                        ys = lt.tile([P, D], F32, tag=f"y{bb}{m}", name=f"y{bb}{m}")
                        for n in range(2):
                            ps = psc.tile([P, 1024], F32, tag="sc", name="sc")
                            for k in range(KT):
                                nc.tensor.matmul(
                                    ps[:, 0:512],
                                    ao[bb * KT + k][:, m * P:(m + 1) * P],
                                    wo[k][:, n * 512:(n + 1) * 512],
                                    start=(k == 0), stop=(k == KT - 1))
                            nc.vector.tensor_tensor(
                                ys[:, n * 512:(n + 1) * 512], ps[:, 0:512],
                                bob[:, n * 512:(n + 1) * 512], ADD)
                        nc.sync.dma_start(d_y[bb, m * P:(m + 1) * P, :], ys[:])

    nc.compile()
    return nc


def get_nc():
    if "nc" not in _CACHE:
        _CACHE["nc"] = _build_nc()
    return _CACHE["nc"]


def make_in_maps(x, Wq, bq, Wk, bk, Wv, bv, Wo, bo):
    x = np.asarray(x, dtype=np.float32)
    Wq, Wk, Wv, Wo = (np.asarray(w, dtype=np.float32) for w in (Wq, Wk, Wv, Wo))
    bq, bk, bv, bo = (np.asarray(v, dtype=np.float32) for v in (bq, bk, bv, bo))
    scale = 1.0 / np.sqrt(np.float32(HD))
    woT = np.ascontiguousarray(Wo.T)
    bo1 = bo.reshape(1, D)
    in_maps = []
    for c in range(CORES):
        b, g = c // 4, c % 4
        sl = slice(g * LD, (g + 1) * LD)
        xT = np.ascontiguousarray(x[b].T)
        wqT = np.ascontiguousarray(Wq[sl, :].T) * scale
        wkT = np.ascontiguousarray(Wk[sl, :].T)
        wvT = np.ascontiguousarray(Wv[sl, :].T)
        bq2 = np.ascontiguousarray((bq[sl] * scale).reshape(2, P).T)
        bk2 = np.ascontiguousarray(bk[sl].reshape(2, P).T)
        vinit = np.empty((1, GH * (HD + 1)), dtype=np.float32)
        for h in range(GH):
            vinit[0, h * (HD + 1): h * (HD + 1) + HD] = bv[g * LD + h * HD:
                                                           g * LD + (h + 1) * HD]
            vinit[0, h * (HD + 1) + HD] = 1.0
        in_maps.append({
            "xT": xT, "wqT": wqT, "wkT": wkT, "wvT": wvT,
            "bq2": bq2, "bk2": bk2, "vinit": vinit, "woT": woT, "bo1": bo1,
        })
    return in_maps


def assemble(results):
    out = np.empty((B, S, D), dtype=np.float32)
    for c in range(CORES):
        out[:, c * QS:(c + 1) * QS, :] = results[c]["y"]
    return out


def kernel(**inputs):
    from concourse.bass_utils import run_bass_kernel_spmd

    nc = get_nc()
    in_maps = make_in_maps(**inputs)
    res = run_bass_kernel_spmd(nc, in_maps, list(range(CORES)), trace=False)
    return assemble(res.results)


# revision 5
# speedup vs baseline: 1.2158x; 1.2158x over previous
"""MHA (B=2, S=2048, D=1024, H=16) on 8 Trainium2 NeuronCores.

Sharding: core c = (batch b = c//4, head-group g = c%4, 4 heads each).
Per core: Q/K/V projections for its 4 heads (tensor-parallel column split),
full attention for those heads in transposed (scoresT = [k, q]) layout so no
on-chip transposes are needed, then an 8-way AllToAll redistributes the
per-head outputs so each core owns a 256-row sequence slice of BOTH batches
with all 1024 concat-head dims, and runs the output projection for its slice.
Host only concatenates slices.

All matmuls run as float32r (FP22 truncated reads — full PE rate, ~1e-4 rel
err). Softmax skips max-subtraction (scores ~N(0,1), fp32 exp is safe) and
folds the row-sum into the attention@V matmul via an extra all-ones column on
V; normalization happens after, via partition_broadcast + reciprocal.
"""

import numpy as np

B, S, D, H = 2, 2048, 1024, 16
HD = D // H          # 64
G = H // 4           # 4 head-groups
GH = 4               # heads per core
LD = GH * HD         # 256 local dims per core
CORES = 8
QS = S // CORES      # 256: per-core final sequence slice (per batch)
P = 128
NB = 4               # 2048 / 512 column blocks
KT = D // P          # 8 k-tiles of the model dim

_CACHE = {}


def _build_nc():
    import concourse.mybir as mybir
    import concourse.tile as tile
    from concourse import bacc

    F32 = mybir.dt.float32
    F32R = mybir.dt.float32r
    EXP = mybir.ActivationFunctionType.Exp
    MUL = mybir.AluOpType.mult
    ADD = mybir.AluOpType.add

    nc = bacc.Bacc("TRN2", target_bir_lowering=False, debug=False,
                   num_devices=CORES)

    d_xT = nc.dram_tensor("xT", [D, S], F32, kind="ExternalInput")
    d_wq = nc.dram_tensor("wqT", [D, LD], F32, kind="ExternalInput")
    d_wk = nc.dram_tensor("wkT", [D, LD], F32, kind="ExternalInput")
    d_wv = nc.dram_tensor("wvT", [D, LD], F32, kind="ExternalInput")
    d_bq = nc.dram_tensor("bq2", [P, 2], F32, kind="ExternalInput")
    d_bk = nc.dram_tensor("bk2", [P, 2], F32, kind="ExternalInput")
    d_vi = nc.dram_tensor("vinit", [1, GH * (HD + 1)], F32, kind="ExternalInput")
    d_wo = nc.dram_tensor("woT", [D, D], F32, kind="ExternalInput")
    d_bo = nc.dram_tensor("bo1", [1, D], F32, kind="ExternalInput")
    d_y = nc.dram_tensor("y", [B, QS, D], F32, kind="ExternalOutput")

    VW = HD + 1  # 65: per-head V width incl. ones column

    with tile.TileContext(nc) as tc:
        with (
            tc.tile_pool(name="statics", bufs=1) as st,
            tc.tile_pool(name="dram", bufs=1, space="DRAM") as dram,
        ):
            # ---- small persistent statics ----
            bq = st.tile([P, 2], F32, tag="bq", name="bq")
            bk = st.tile([P, 2], F32, tag="bk", name="bk")
            nc.sync.dma_start(bq[:], d_bq[:])
            nc.sync.dma_start(bk[:], d_bk[:])
            vib = st.tile([P, GH * VW], F32, tag="vib", name="vib")
            bob = st.tile([P, D], F32, tag="bob", name="bob")

            # persistent activations
            qT = [st.tile([P, S], F32R, tag=f"qT{t}", name=f"qT{t}") for t in range(2)]
            kTt = [st.tile([P, S], F32R, tag=f"kT{t}", name=f"kT{t}") for t in range(2)]
            vaug = [st.tile([P, GH * VW], F32R, tag=f"va{s}", name=f"va{s}") for s in range(S // P)]
            outT = [st.tile([P, S], F32R, tag=f"oT{t}", name=f"oT{t}") for t in range(2)]

            with (
                tc.tile_pool(name="xTp", bufs=1) as xp,
                tc.tile_pool(name="pj", bufs=3, space="PSUM") as pj,
                tc.tile_pool(name="pv", bufs=2, space="PSUM") as pv,
            ):
                wq = [xp.tile([P, LD], F32R, tag=f"wq{k}", name=f"wq{k}") for k in range(KT)]
                wk = [xp.tile([P, LD], F32R, tag=f"wk{k}", name=f"wk{k}") for k in range(KT)]
                wv = [xp.tile([P, LD], F32R, tag=f"wv{k}", name=f"wv{k}") for k in range(KT)]
                for k in range(KT):
                    nc.sync.dma_start(wq[k][:], d_wq[k * P:(k + 1) * P, :].bitcast(F32R))
                    nc.sync.dma_start(wk[k][:], d_wk[k * P:(k + 1) * P, :].bitcast(F32R))
                    nc.sync.dma_start(wv[k][:], d_wv[k * P:(k + 1) * P, :].bitcast(F32R))
                vi1 = xp.tile([1, GH * VW], F32, tag="vi1", name="vi1")
                nc.sync.dma_start(vi1[:], d_vi[:])
                nc.gpsimd.partition_broadcast(vib[:], vi1[:])
                bo1 = xp.tile([1, D], F32, tag="bo1", name="bo1")
                nc.sync.dma_start(bo1[:], d_bo[:])
                nc.gpsimd.partition_broadcast(bob[:], bo1[:])
                xT = [xp.tile([P, S], F32R, tag=f"xT{k}", name=f"xT{k}") for k in range(KT)]
                for k in range(KT):
                    nc.sync.dma_start(xT[k][:], d_xT[k * P:(k + 1) * P, :].bitcast(F32R))

                # ---- Q/K projections (transposed layout [256, 2048]) ----
                for (ws, bs, dst) in ((wq, bq, qT), (wk, bk, kTt)):
                    for m in range(2):
                        for n in range(NB):
                            ps = pj.tile([P, 512], F32, tag="pj", name="pj")
                            for k in range(KT):
                                nc.tensor.matmul(
                                    ps[:], ws[k][:, m * P:(m + 1) * P],
                                    xT[k][:, n * 512:(n + 1) * 512],
                                    start=(k == 0), stop=(k == KT - 1))
                            nc.vector.tensor_tensor(
                                dst[m][:, n * 512:(n + 1) * 512], ps[:],
                                bs[:, m:m + 1].to_broadcast((P, 512)), ADD)

                # ---- V projection (natural layout, head-interleaved + ones col) ----
                for s in range(S // P):
                    ps = pv.tile([P, LD], F32, tag="pv", name="pv")
                    for k in range(KT):
                        nc.tensor.matmul(
                            ps[:], xT[k][:, s * P:(s + 1) * P], wv[k][:],
                            start=(k == 0), stop=(k == KT - 1))
                    va = vaug[s].rearrange("p (h w) -> p h w", w=VW)
                    vb = vib.rearrange("p (h w) -> p h w", w=VW)
                    nc.vector.tensor_tensor(
                        va[:, :, 0:HD], ps.rearrange("p (h w) -> p h w", w=HD),
                        vb[:, :, 0:HD], ADD)
                    nc.vector.tensor_copy(va[:, :, HD:VW], vb[:, :, HD:VW])

            # ---- attention, per head, scoresT = [k_pos, q_pos] layout ----
            with (
                tc.tile_pool(name="att", bufs=3) as at,
                tc.tile_pool(name="nrm", bufs=1) as nr,
                tc.tile_pool(name="psc", bufs=2, space="PSUM") as psc,
                tc.tile_pool(name="pav", bufs=1, space="PSUM") as pav,
                tc.tile_pool(name="late", bufs=1) as lt,
            ):
                wo = [lt.tile([P, D], F32R, tag=f"wo{k}", name=f"wo{k}") for k in range(KT)]
                for k in range(KT):
                    nc.sync.dma_start(wo[k][:], d_wo[k * P:(k + 1) * P, :].bitcast(F32R))

                a_ins = [dram.tile([CORES * HD, QS], F32, name=f"a_in{h}")
                         for h in range(GH)]
                a_outs = [dram.tile([CORES * HD, QS], F32, name=f"a_out{h}")
                          for h in range(GH)]
                for h in range(GH):
                    ht, hp = h // 2, (h % 2) * HD
                    kTh = kTt[ht][hp:hp + HD, :]
                    qTh = qT[ht][hp:hp + HD, :]
                    av = pav.tile([VW, S], F32, tag="av", name="av")
                    for kb in range(S // P):
                        ex = at.tile([P, S], F32R, tag="ex", name="ex")
                        for half in range(2):
                            sc = psc.tile([P, 1024], F32, tag="sc", name="sc")
                            for nn in range(2):
                                qc = half * 1024 + nn * 512
                                nc.tensor.matmul(
                                    sc[:, nn * 512:(nn + 1) * 512],
                                    kTh[:, kb * P:(kb + 1) * P],
                                    qTh[:, qc:qc + 512],
                                    start=True, stop=True)
                            nc.scalar.activation(
                                ex[:, half * 1024:(half + 1) * 1024], sc[:], EXP)
                        for qb in range(NB):
                            nc.tensor.matmul(
                                av[:, qb * 512:(qb + 1) * 512],
                                vaug[kb][:, h * VW:(h + 1) * VW],
                                ex[:, qb * 512:(qb + 1) * 512],
                                start=(kb == 0), stop=(kb == S // P - 1))
                    # normalize: outT_h = av[0:64] / av[64]
                    rs = nr.tile([1, S], F32, tag="rs", name="rs")
                    nc.vector.tensor_copy(rs[:], av[HD:VW, :])
                    rsb = nr.tile([HD, S], F32, tag="rsb", name="rsb")
                    nc.gpsimd.partition_broadcast(rsb[:], rs[:])
                    nc.vector.reciprocal(rsb[:], rsb[:])
                    nc.vector.tensor_tensor(
                        outT[ht][hp:hp + HD, :], av[0:HD, :], rsb[:], MUL)

                    # per-head 8-way AllToAll (overlaps remaining heads)
                    for j in range(CORES):
                        nc.sync.dma_start(
                            a_ins[h][j * HD:(j + 1) * HD, :],
                            outT[ht][hp:hp + HD, j * QS:(j + 1) * QS].bitcast(F32))
                    nc.gpsimd.collective_compute(
                        "AllToAll",
                        mybir.AluOpType.bypass,
                        replica_groups=[list(range(CORES))],
                        ins=[a_ins[h][:]],
                        outs=[a_outs[h][:]],
                    )

                # ---- out projection for my 256-row slice of each batch ----
                # SBUF k-tile i = (bb, k): dims [k*128,(k+1)*128) of batch bb =
                # group g'=k//2, heads 2*(k%2), 2*(k%2)+1; rows (bb*4+g')*64 of
                # those heads' a_out tensors.
                ao = [lt.tile([P, QS], F32R, tag=f"ao{i}", name=f"ao{i}") for i in range(2 * KT)]
                for bb in range(B):
                    for k in range(KT):
                        gp, h0 = k // 2, 2 * (k % 2)
                        r0 = (bb * 4 + gp) * HD
                        i = bb * KT + k
                        nc.sync.dma_start(
                            ao[i][0:HD, :],
                            a_outs[h0][r0:r0 + HD, :].bitcast(F32R))
                        nc.sync.dma_start(
                            ao[i][HD:P, :],
                            a_outs[h0 + 1][r0:r0 + HD, :].bitcast(F32R))
                for bb in range(B):
                    for m in range(QS // P):
                        ys = lt.tile([P, D], F32, tag=f"y{bb}{m}", name=f"y{bb}{m}")
                        for n in range(2):
                            ps = psc.tile([P, 1024], F32, tag="sc", name="sc")
                            # heads-0/1 k-tiles (even k) first so accumulation
                            # can start before heads 2/3 finish their A2A
                            korder = [k for k in range(KT) if k % 2 == 0] + \
                                     [k for k in range(KT) if k % 2 == 1]
                            for ki, k in enumerate(korder):
                                nc.tensor.matmul(
                                    ps[:, 0:512],
                                    ao[bb * KT + k][:, m * P:(m + 1) * P],
                                    wo[k][:, n * 512:(n + 1) * 512],
                                    start=(ki == 0), stop=(ki == KT - 1))
                            nc.vector.tensor_tensor(
                                ys[:, n * 512:(n + 1) * 512], ps[:, 0:512],
                                bob[:, n * 512:(n + 1) * 512], ADD)
                        nc.sync.dma_start(d_y[bb, m * P:(m + 1) * P, :], ys[:])

    nc.compile()
    return nc


def get_nc():
    if "nc" not in _CACHE:
        _CACHE["nc"] = _build_nc()
    return _CACHE["nc"]


def make_in_maps(x, Wq, bq, Wk, bk, Wv, bv, Wo, bo):
    x = np.asarray(x, dtype=np.float32)
    Wq, Wk, Wv, Wo = (np.asarray(w, dtype=np.float32) for w in (Wq, Wk, Wv, Wo))
    bq, bk, bv, bo = (np.asarray(v, dtype=np.float32) for v in (bq, bk, bv, bo))
    scale = 1.0 / np.sqrt(np.float32(HD))
    woT = np.ascontiguousarray(Wo.T)
    bo1 = bo.reshape(1, D)
    in_maps = []
    for c in range(CORES):
        b, g = c // 4, c % 4
        sl = slice(g * LD, (g + 1) * LD)
        xT = np.ascontiguousarray(x[b].T)
        wqT = np.ascontiguousarray(Wq[sl, :].T) * scale
        wkT = np.ascontiguousarray(Wk[sl, :].T)
        wvT = np.ascontiguousarray(Wv[sl, :].T)
        bq2 = np.ascontiguousarray((bq[sl] * scale).reshape(2, P).T)
        bk2 = np.ascontiguousarray(bk[sl].reshape(2, P).T)
        vinit = np.empty((1, GH * (HD + 1)), dtype=np.float32)
        for h in range(GH):
            vinit[0, h * (HD + 1): h * (HD + 1) + HD] = bv[g * LD + h * HD:
                                                           g * LD + (h + 1) * HD]
            vinit[0, h * (HD + 1) + HD] = 1.0
        in_maps.append({
            "xT": xT, "wqT": wqT, "wkT": wkT, "wvT": wvT,
            "bq2": bq2, "bk2": bk2, "vinit": vinit, "woT": woT, "bo1": bo1,
        })
    return in_maps


def assemble(results):
    out = np.empty((B, S, D), dtype=np.float32)
    for c in range(CORES):
        out[:, c * QS:(c + 1) * QS, :] = results[c]["y"]
    return out


def kernel(**inputs):
    from concourse.bass_utils import run_bass_kernel_spmd

    nc = get_nc()
    in_maps = make_in_maps(**inputs)
    res = run_bass_kernel_spmd(nc, in_maps, list(range(CORES)), trace=False)
    return assemble(res.results)


# revision 6
# speedup vs baseline: 318.9161x; 262.3010x over previous
"""MHA (B=2, S=2048, D=1024, H=16) on 8 Trainium2 NeuronCores.

Sharding: core c = (batch b = c//4, head-group g = c%4, 4 heads each).
Per core: Q/K/V projections for its 4 heads (tensor-parallel column split),
full attention for those heads in transposed (scoresT = [k, q]) layout so no
on-chip transposes are needed, then an 8-way AllToAll redistributes the
per-head outputs so each core owns a 256-row sequence slice of BOTH batches
with all 1024 concat-head dims, and runs the output projection for its slice.
Host only concatenates slices.

All matmuls run as float32r (FP22 truncated reads — full PE rate, ~1e-4 rel
err). Softmax skips max-subtraction (scores ~N(0,1), fp32 exp is safe) and
folds the row-sum into the attention@V matmul via an extra all-ones column on
V; normalization happens after, via partition_broadcast + reciprocal.
"""

import numpy as np

B, S, D, H = 2, 2048, 1024, 16
HD = D // H          # 64
G = H // 4           # 4 head-groups
GH = 4               # heads per core
LD = GH * HD         # 256 local dims per core
CORES = 8
QS = S // CORES      # 256: per-core final sequence slice (per batch)
P = 128
NB = 4               # 2048 / 512 column blocks
KT = D // P          # 8 k-tiles of the model dim

_CACHE = {}


def _build_nc():
    import concourse.mybir as mybir
    import concourse.tile as tile
    from concourse import bacc

    F32 = mybir.dt.float32
    F32R = mybir.dt.float32r
    EXP = mybir.ActivationFunctionType.Exp
    MUL = mybir.AluOpType.mult
    ADD = mybir.AluOpType.add

    nc = bacc.Bacc("TRN2", target_bir_lowering=False, debug=False,
                   num_devices=CORES)

    d_xT = nc.dram_tensor("xT", [D, S], F32, kind="ExternalInput")
    d_wq = nc.dram_tensor("wqT", [D, LD], F32, kind="ExternalInput")
    d_wk = nc.dram_tensor("wkT", [D, LD], F32, kind="ExternalInput")
    d_wv = nc.dram_tensor("wvT", [D, LD], F32, kind="ExternalInput")
    d_bq = nc.dram_tensor("bq2", [P, 2], F32, kind="ExternalInput")
    d_bk = nc.dram_tensor("bk2", [P, 2], F32, kind="ExternalInput")
    d_vi = nc.dram_tensor("vinit", [1, GH * (HD + 1)], F32, kind="ExternalInput")
    d_wo = nc.dram_tensor("woT", [D, D], F32, kind="ExternalInput")
    d_bo = nc.dram_tensor("bo1", [1, D], F32, kind="ExternalInput")
    d_y = nc.dram_tensor("y", [B, QS, D], F32, kind="ExternalOutput")

    VW = HD + 1  # 65: per-head V width incl. ones column

    with tile.TileContext(nc) as tc:
        with (
            tc.tile_pool(name="statics", bufs=1) as st,
            tc.tile_pool(name="dram", bufs=1, space="DRAM") as dram,
        ):
            # ---- small persistent statics ----
            bq = st.tile([P, 2], F32, tag="bq", name="bq")
            bk = st.tile([P, 2], F32, tag="bk", name="bk")
            nc.sync.dma_start(bq[:], d_bq[:])
            nc.sync.dma_start(bk[:], d_bk[:])
            vib = st.tile([P, GH * VW], F32, tag="vib", name="vib")
            bob = st.tile([P, D], F32, tag="bob", name="bob")

            # persistent activations
            qT = [st.tile([P, S], F32R, tag=f"qT{t}", name=f"qT{t}") for t in range(2)]
            kTt = [st.tile([P, S], F32R, tag=f"kT{t}", name=f"kT{t}") for t in range(2)]
            vaug = [st.tile([P, GH * VW], F32R, tag=f"va{s}", name=f"va{s}") for s in range(S // P)]
            outT = [st.tile([P, S], F32R, tag=f"oT{t}", name=f"oT{t}") for t in range(2)]

            with (
                tc.tile_pool(name="xTp", bufs=1) as xp,
                tc.tile_pool(name="pj", bufs=3, space="PSUM") as pj,
                tc.tile_pool(name="pv", bufs=2, space="PSUM") as pv,
            ):
                wq = [xp.tile([P, LD], F32R, tag=f"wq{k}", name=f"wq{k}") for k in range(KT)]
                wk = [xp.tile([P, LD], F32R, tag=f"wk{k}", name=f"wk{k}") for k in range(KT)]
                wv = [xp.tile([P, LD], F32R, tag=f"wv{k}", name=f"wv{k}") for k in range(KT)]
                for k in range(KT):
                    nc.sync.dma_start(wq[k][:], d_wq[k * P:(k + 1) * P, :].bitcast(F32R))
                    nc.sync.dma_start(wk[k][:], d_wk[k * P:(k + 1) * P, :].bitcast(F32R))
                    nc.sync.dma_start(wv[k][:], d_wv[k * P:(k + 1) * P, :].bitcast(F32R))
                vi1 = xp.tile([1, GH * VW], F32, tag="vi1", name="vi1")
                nc.sync.dma_start(vi1[:], d_vi[:])
                nc.gpsimd.partition_broadcast(vib[:], vi1[:])
                bo1 = xp.tile([1, D], F32, tag="bo1", name="bo1")
                nc.sync.dma_start(bo1[:], d_bo[:])
                nc.gpsimd.partition_broadcast(bob[:], bo1[:])
                xT = [xp.tile([P, S], F32R, tag=f"xT{k}", name=f"xT{k}") for k in range(KT)]
                for k in range(KT):
                    nc.sync.dma_start(xT[k][:], d_xT[k * P:(k + 1) * P, :].bitcast(F32R))

                # ---- Q/K projections (transposed layout [256, 2048]) ----
                for (ws, bs, dst) in ((wq, bq, qT), (wk, bk, kTt)):
                    for m in range(2):
                        for n in range(NB):
                            ps = pj.tile([P, 512], F32, tag="pj", name="pj")
                            for k in range(KT):
                                nc.tensor.matmul(
                                    ps[:], ws[k][:, m * P:(m + 1) * P],
                                    xT[k][:, n * 512:(n + 1) * 512],
                                    start=(k == 0), stop=(k == KT - 1))
                            nc.vector.tensor_tensor(
                                dst[m][:, n * 512:(n + 1) * 512], ps[:],
                                bs[:, m:m + 1].to_broadcast((P, 512)), ADD)

                # ---- V projection (natural layout, head-interleaved + ones col) ----
                for s in range(S // P):
                    ps = pv.tile([P, LD], F32, tag="pv", name="pv")
                    for k in range(KT):
                        nc.tensor.matmul(
                            ps[:], xT[k][:, s * P:(s + 1) * P], wv[k][:],
                            start=(k == 0), stop=(k == KT - 1))
                    va = vaug[s].rearrange("p (h w) -> p h w", w=VW)
                    vb = vib.rearrange("p (h w) -> p h w", w=VW)
                    nc.vector.tensor_tensor(
                        va[:, :, 0:HD], ps.rearrange("p (h w) -> p h w", w=HD),
                        vb[:, :, 0:HD], ADD)
                    nc.vector.tensor_copy(va[:, :, HD:VW], vb[:, :, HD:VW])

            # ---- attention, per head, scoresT = [k_pos, q_pos] layout ----
            with (
                tc.tile_pool(name="att", bufs=3) as at,
                tc.tile_pool(name="nrm", bufs=1) as nr,
                tc.tile_pool(name="psc", bufs=2, space="PSUM") as psc,
                tc.tile_pool(name="pav", bufs=1, space="PSUM") as pav,
                tc.tile_pool(name="late", bufs=1) as lt,
            ):
                wo = [lt.tile([P, D], F32R, tag=f"wo{k}", name=f"wo{k}") for k in range(KT)]
                for k in range(KT):
                    nc.sync.dma_start(wo[k][:], d_wo[k * P:(k + 1) * P, :].bitcast(F32R))

                a_ins = [dram.tile([CORES * HD, QS], F32, name=f"a_in{h}")
                         for h in range(GH)]
                a_outs = [dram.tile([CORES * HD, QS], F32, name=f"a_out{h}")
                          for h in range(GH)]
                for h in range(GH):
                    ht, hp = h // 2, (h % 2) * HD
                    kTh = kTt[ht][hp:hp + HD, :]
                    qTh = qT[ht][hp:hp + HD, :]
                    av = pav.tile([VW, S], F32, tag="av", name="av")
                    for kb in range(S // P):
                        ex = at.tile([P, S], F32R, tag="ex", name="ex")
                        for half in range(2):
                            sc = psc.tile([P, 1024], F32, tag="sc", name="sc")
                            for nn in range(2):
                                qc = half * 1024 + nn * 512
                                nc.tensor.matmul(
                                    sc[:, nn * 512:(nn + 1) * 512],
                                    kTh[:, kb * P:(kb + 1) * P],
                                    qTh[:, qc:qc + 512],
                                    start=True, stop=True)
                            nc.scalar.activation(
                                ex[:, half * 1024:(half + 1) * 1024], sc[:], EXP)
                        for qb in range(NB):
                            nc.tensor.matmul(
                                av[:, qb * 512:(qb + 1) * 512],
                                vaug[kb][:, h * VW:(h + 1) * VW],
                                ex[:, qb * 512:(qb + 1) * 512],
                                start=(kb == 0), stop=(kb == S // P - 1))
                    # normalize: outT_h = av[0:64] / av[64]; copy PSUM out
                    # first so the next head's AV matmuls get the banks early
                    rs = nr.tile([1, S], F32, tag="rs", name="rs")
                    nc.vector.tensor_copy(rs[:], av[HD:VW, :])
                    avs = nr.tile([HD, S], F32, tag="avs", name="avs")
                    nc.vector.tensor_copy(avs[:], av[0:HD, :])
                    rsb = nr.tile([HD, S], F32, tag="rsb", name="rsb")
                    nc.gpsimd.partition_broadcast(rsb[:], rs[:])
                    nc.vector.reciprocal(rsb[:], rsb[:])
                    nc.vector.tensor_tensor(
                        outT[ht][hp:hp + HD, :], avs[:], rsb[:], MUL)

                    # per-head 8-way AllToAll (overlaps remaining heads)
                    for j in range(CORES):
                        nc.sync.dma_start(
                            a_ins[h][j * HD:(j + 1) * HD, :],
                            outT[ht][hp:hp + HD, j * QS:(j + 1) * QS].bitcast(F32))
                    nc.gpsimd.collective_compute(
                        "AllToAll",
                        mybir.AluOpType.bypass,
                        replica_groups=[list(range(CORES))],
                        ins=[a_ins[h][:]],
                        outs=[a_outs[h][:]],
                    )

                # ---- out projection for my 256-row slice of each batch ----
                # SBUF k-tile i = (bb, k): dims [k*128,(k+1)*128) of batch bb =
                # group g'=k//2, heads 2*(k%2), 2*(k%2)+1; rows (bb*4+g')*64 of
                # those heads' a_out tensors.
                ao = [lt.tile([P, QS], F32R, tag=f"ao{i}", name=f"ao{i}") for i in range(2 * KT)]
                for bb in range(B):
                    for k in range(KT):
                        gp, h0 = k // 2, 2 * (k % 2)
                        r0 = (bb * 4 + gp) * HD
                        i = bb * KT + k
                        nc.sync.dma_start(
                            ao[i][0:HD, :],
                            a_outs[h0][r0:r0 + HD, :].bitcast(F32R))
                        nc.sync.dma_start(
                            ao[i][HD:P, :],
                            a_outs[h0 + 1][r0:r0 + HD, :].bitcast(F32R))
                for bb in range(B):
                    for m in range(QS // P):
                        ys = lt.tile([P, D], F32, tag=f"y{bb}{m}", name=f"y{bb}{m}")
                        for n in range(2):
                            ps = psc.tile([P, 1024], F32, tag="sc", name="sc")
                            # heads-0/1 k-tiles (even k) first so accumulation
                            # can start before heads 2/3 finish their A2A
                            korder = [k for k in range(KT) if k % 2 == 0] + \
                                     [k for k in range(KT) if k % 2 == 1]
                            for ki, k in enumerate(korder):
                                nc.tensor.matmul(
                                    ps[:, 0:512],
                                    ao[bb * KT + k][:, m * P:(m + 1) * P],
                                    wo[k][:, n * 512:(n + 1) * 512],
                                    start=(ki == 0), stop=(ki == KT - 1))
                            nc.vector.tensor_tensor(
                                ys[:, n * 512:(n + 1) * 512], ps[:, 0:512],
                                bob[:, n * 512:(n + 1) * 512], ADD)
                        nc.sync.dma_start(d_y[bb, m * P:(m + 1) * P, :], ys[:])

    nc.compile()
    return nc


def get_nc():
    if "nc" not in _CACHE:
        _CACHE["nc"] = _build_nc()
    return _CACHE["nc"]


def make_in_maps(x, Wq, bq, Wk, bk, Wv, bv, Wo, bo):
    x = np.asarray(x, dtype=np.float32)
    Wq, Wk, Wv, Wo = (np.asarray(w, dtype=np.float32) for w in (Wq, Wk, Wv, Wo))
    bq, bk, bv, bo = (np.asarray(v, dtype=np.float32) for v in (bq, bk, bv, bo))
    scale = 1.0 / np.sqrt(np.float32(HD))
    woT = np.ascontiguousarray(Wo.T)
    bo1 = bo.reshape(1, D)
    in_maps = []
    for c in range(CORES):
        b, g = c // 4, c % 4
        sl = slice(g * LD, (g + 1) * LD)
        xT = np.ascontiguousarray(x[b].T)
        wqT = np.ascontiguousarray(Wq[sl, :].T) * scale
        wkT = np.ascontiguousarray(Wk[sl, :].T)
        wvT = np.ascontiguousarray(Wv[sl, :].T)
        bq2 = np.ascontiguousarray((bq[sl] * scale).reshape(2, P).T)
        bk2 = np.ascontiguousarray(bk[sl].reshape(2, P).T)
        vinit = np.empty((1, GH * (HD + 1)), dtype=np.float32)
        for h in range(GH):
            vinit[0, h * (HD + 1): h * (HD + 1) + HD] = bv[g * LD + h * HD:
                                                           g * LD + (h + 1) * HD]
            vinit[0, h * (HD + 1) + HD] = 1.0
        in_maps.append({
            "xT": xT, "wqT": wqT, "wkT": wkT, "wvT": wvT,
            "bq2": bq2, "bk2": bk2, "vinit": vinit, "woT": woT, "bo1": bo1,
        })
    return in_maps


def assemble(results):
    out = np.empty((B, S, D), dtype=np.float32)
    for c in range(CORES):
        out[:, c * QS:(c + 1) * QS, :] = results[c]["y"]
    return out


def kernel(**inputs):
    from concourse.bass_utils import run_bass_kernel_spmd

    nc = get_nc()
    in_maps = make_in_maps(**inputs)
    res = run_bass_kernel_spmd(nc, in_maps, list(range(CORES)), trace=False)
    return assemble(res.results)
